# revision 14
# baseline (speedup 1.0000x reference)
"""ViT-Base forward on 8 TRN2 NeuronCores, data-parallel over batch (4 images/core).

Layout: token-major fp32 residual x in 8 per-image tiles ([128]+[69 real] rows per
image, cls token LAST at row 68 of the odd tile). bf16 matmuls everywhere, fp32
accumulation. Feature-major activations (hT/qT/kT) produced by bf16 DMA-transpose.
Per-head QKV as block-diagonal [128,128] matmuls. Softmax without max-subtraction
(scores are O(0.1) for this model family). LN gamma/beta, attention scale, and all
biases are folded host-side into weights / bias rows.
"""
import os
import numpy as np
import ml_dtypes
from contextlib import ExitStack

import concourse.bass as bass
from concourse import bacc
import concourse.tile as tile
import concourse.mybir as mybir
from concourse.bass_utils import run_bass_kernel_spmd

BF = ml_dtypes.bfloat16
F32 = mybir.dt.float32
BF16 = mybir.dt.bfloat16
AF = mybir.ActivationFunctionType
ALU = mybir.AluOpType

P = 128
L, H, DH, D = 12, 12, 64, 768
NPATCH = 196            # patches per image
S = NPATCH + 1          # 197 tokens per image
BIMG = 4                # images per core
NCORES = 8
DK = 6                  # 768/128 k-tiles
MK = 24                 # 3072/128 k-tiles
HID = 3072
EPS = 1e-5

# per-image token rows: tile A = tokens 0..127 (patches), tile B rows 0..68
# (patches 128..195 then cls at row 68). 8 x-tiles total, sizes [128,69]*4.
TLEN = [128, 69]


def _pos_embed():
    i = np.arange(S, dtype=np.float64)[:, None]
    j = np.arange(D)
    angle = i / np.power(10000.0, (2.0 * (j // 2)) / D)
    return np.where(j % 2 == 0, np.sin(angle), np.cos(angle)).astype(np.float32)


def _build():
    nc = bacc.Bacc("TRN2", target_bir_lowering=False, debug=False)
    dt = nc.dram_tensor
    patchesT = dt("patchesT", [DK, P, BIMG * NPATCH], BF16, kind="ExternalInput")
    base_x = dt("base_x", [8, P, D], F32, kind="ExternalInput")
    WeT = dt("WeT", [DK, P, D], BF16, kind="ExternalInput")
    be_row = dt("be_row", [1, D], BF16, kind="ExternalInput")
    Wqk_d = dt("Wqk_d", [L, 2, DK, P, P], BF16, kind="ExternalInput")
    Wv_d = dt("Wv_d", [L, DK, P, P], BF16, kind="ExternalInput")
    bqkv_d = dt("bqkv_d", [L, 3, DK, P], F32, kind="ExternalInput")
    W1T_d = dt("W1T_d", [L, DK, P, HID], BF16, kind="ExternalInput")
    b1_d = dt("b1_d", [L, MK, P], F32, kind="ExternalInput")
    W2T_d = dt("W2T_d", [L, MK, P, D], BF16, kind="ExternalInput")
    b2_d = dt("b2_d", [L, 1, D], BF16, kind="ExternalInput")
    bv_row_d = dt("bv_row_d", [L, 1, D], BF16, kind="ExternalInput")
    WhT_d = dt("WhT_d", [DK, P, 1000], BF16, kind="ExternalInput")
    bh_d = dt("bh_d", [1, 1000], BF16, kind="ExternalInput")
    out_d = dt("out", [BIMG, 1000], F32, kind="ExternalOutput")

    with tile.TileContext(nc) as tc, ExitStack() as ctx:
        pool = ctx.enter_context(tc.tile_pool(name="sb", bufs=1))
        wpool = ctx.enter_context(tc.tile_pool(name="wp", bufs=1))
        psum = ctx.enter_context(tc.tile_pool(name="ps", bufs=1, space="PSUM"))

        ones_b = pool.tile([1, P], BF16)
        nc.vector.memset(ones_b[:], 1.0)
        from concourse.masks import make_identity
        ident_b = pool.tile([P, P], BF16)
        make_identity(nc, ident_b)
        eps_t = pool.tile([P, 1], F32)
        nc.vector.memset(eps_t[:], EPS)

        # persistent residual tiles, preloaded with pos+cls base
        xt = []
        for t in range(8):
            x_tile = pool.tile([P, D], F32, tag=f"x{t}", name=f"x{t}")
            nc.sync.dma_start(x_tile[:], base_x[t])
            xt.append(x_tile)

        # ---- embed: x += patchesT.T @ WeT (+ be) ----
        with ExitStack() as ectx:
            epool = ectx.enter_context(tc.tile_pool(name="ep", bufs=1))
            patT = epool.tile([P, DK, BIMG * NPATCH], BF16)
            nc.sync.dma_start(patT[:], patchesT.rearrange("k p m -> p k m"))
            weT = epool.tile([P, DK, D], BF16)
            nc.sync.dma_start(weT[:], WeT.rearrange("k p m -> p k m"))
            berow = epool.tile([1, D], BF16)
            nc.sync.dma_start(berow[:], be_row[:, :])
            for i in range(BIMG):
                for st in range(2):
                    slen = 128 if st == 0 else 68  # patches only; cls row untouched
                    c0 = NPATCH * i + st * 128
                    for hf in range(2):
                        ps = psum.tile([P, 384], F32, tag="A", bufs=2)
                        for k in range(DK):
                            nc.tensor.matmul(ps[:slen], patT[:, k, c0:c0 + slen],
                                             weT[:, k, hf * 384:(hf + 1) * 384],
                                             start=(k == 0), stop=False)
                        nc.tensor.matmul(ps[:slen], ones_b[:, :slen],
                                         berow[:, hf * 384:(hf + 1) * 384],
                                         start=False, stop=True)
                        xs = xt[2 * i + st][:slen, hf * 384:(hf + 1) * 384]
                        nc.vector.tensor_tensor(xs, ps[:slen], xs, ALU.add)

        # ---- transformer layers ----
        def load_weights(l):
            w = {}
            w["w1t"] = wpool.tile([P, DK, HID], BF16, tag="w1t", name="w1t")
            nc.sync.dma_start(w["w1t"][:], W1T_d[l].rearrange("k p m -> p k m"))
            w["w2t"] = wpool.tile([P, MK, D], BF16, tag="w2t", name="w2t")
            nc.sync.dma_start(w["w2t"][:], W2T_d[l].rearrange("k p m -> p k m"))
            w["wqk"] = wpool.tile([P, 2, DK, P], BF16, tag="wqk", name="wqk")
            for j in range(2):
                nc.sync.dma_start(w["wqk"][:, j], Wqk_d[l, j].rearrange("t p m -> p t m"))
            w["wv"] = wpool.tile([P, DK, P], BF16, tag="wv", name="wv")
            nc.sync.dma_start(w["wv"][:], Wv_d[l].rearrange("t p m -> p t m"))
            w["bqkv"] = wpool.tile([P, 3, DK], F32, tag="bqkv", name="bqkv")
            nc.sync.dma_start(w["bqkv"][:], bqkv_d[l].rearrange("a t p -> p a t"))
            w["b1p"] = wpool.tile([P, MK], F32, tag="b1p", name="b1p")
            nc.sync.dma_start(w["b1p"][:], b1_d[l].rearrange("a p -> p a"))
            w["b2row"] = wpool.tile([1, D], BF16, tag="b2row", name="b2row")
            nc.sync.dma_start(w["b2row"][:], b2_d[l, :, :])
            w["bvrow"] = wpool.tile([1, D], BF16, tag="bvrow", name="bvrow")
            nc.sync.dma_start(w["bvrow"][:], bv_row_d[l, :, :])
            return w

        wcur = load_weights(0)
        for l in range(L):
            w1t, w2t, wqk = wcur["w1t"], wcur["w2t"], wcur["wqk"]
            wv, bqkv, b1p = wcur["wv"], wcur["bqkv"], wcur["b1p"]
            b2row, bvrow = wcur["b2row"], wcur["bvrow"]

            def layernorm_transpose(tag, split=False):
                """LN each x tile -> bf16 h, DMA-transpose into hT (monolithic or per-image)."""
                if split:
                    hTs = [pool.tile([P, DK, 2 * P], BF16, tag="hTs", bufs=6,
                                     name="hTs") for _ in range(BIMG)]
                else:
                    hT = pool.tile([P, DK, 8 * P], BF16, tag="hT", bufs=1, name="hT")
                for t in range(8):
                    st = pool.tile([P, 2, 6], F32, tag="st", bufs=2, name="st")
                    nc.vector.bn_stats(st[:, 0], xt[t][:, 0:384])
                    nc.vector.bn_stats(st[:, 1], xt[t][:, 384:768])
                    ag = pool.tile([P, 2], F32, tag="ag", bufs=2, name="ag")
                    nc.vector.bn_aggr(ag[:], st[:])
                    sd = pool.tile([P, 1], F32, tag="sd", bufs=2, name="sd")
                    nc.scalar.activation(sd[:], ag[:, 1:2], AF.Sqrt, bias=eps_t[:])
                    rs = pool.tile([P, 1], F32, tag="rs", bufs=2, name="rs")
                    nc.vector.reciprocal(rs[:], sd[:])
                    nm = pool.tile([P, 1], F32, tag="nm", bufs=2, name="nm")
                    nc.vector.tensor_tensor(nm[:], ag[:, 0:1], rs[:], ALU.mult)
                    nc.vector.tensor_scalar_mul(nm[:], nm[:], -1.0)
                    h = pool.tile([P, D], BF16, tag="h", bufs=3, name="h")
                    nc.scalar.activation(h[:], xt[t][:], AF.Identity,
                                         bias=nm[:], scale=rs[:])
                    if split:
                        dst = hTs[t // 2][:, :, (t % 2) * P:(t % 2) * P + P]
                    else:
                        dst = hT[:, :, t * P:(t + 1) * P]
                    for c in range(DK):
                        tp = psum.tile([P, P], BF16, tag="A", bufs=2, name="tp")
                        nc.tensor.transpose(tp[:], h[:, c * P:(c + 1) * P], ident_b[:])
                        nc.vector.tensor_copy(out=dst[:, c], in_=tp[:])
                return hTs if split else hT

            # ---- attention ----
            hTs = layernorm_transpose("ln1", split=True)
            o_t = [pool.tile([P, D], BF16, tag=f"o{t}", name=f"o{t}") for t in range(8)]
            for i in range(BIMG if not os.environ.get("SKIP_ATTN") else 0):
                hT = hTs[i]
                ci = 0
                qT = pool.tile([P, DK, S], BF16, tag="qT", bufs=2, name="qT")
                kT = pool.tile([P, DK, S], BF16, tag="kT", bufs=2, name="kT")
                for j, dst in ((0, qT), (1, kT)):
                    for t in range(DK):
                        ps = psum.tile([P, S], F32, tag="A", bufs=2)
                        nc.tensor.matmul(ps[:], wqk[:, j, t], hT[:, t, ci:ci + S],
                                         start=True, stop=True)
                        if j == 0:
                            nc.scalar.activation(dst[:, t], ps[:], AF.Identity,
                                                 bias=bqkv[:, j, t:t + 1])
                        else:
                            nc.vector.tensor_scalar_add(dst[:, t], ps[:],
                                                        bqkv[:, j, t:t + 1])
                va = [None, None]
                for tt in range(2):
                    tlen = TLEN[tt]
                    vat = pool.tile([P, H, 65], BF16, tag="vaug", bufs=3, name="vat")
                    nc.vector.memset(vat[:, :, 64:65], 1.0)
                    for t in range(DK):
                        ps = psum.tile([P, P], F32, tag="A", bufs=2)
                        nc.tensor.matmul(ps[:tlen], hT[:, t, ci + tt * P:ci + tt * P + tlen],
                                         wv[:, t], start=True, stop=False)
                        nc.tensor.matmul(ps[:tlen], ones_b[:, :tlen],
                                         bvrow[:, t * P:(t + 1) * P],
                                         start=False, stop=True)
                        nc.vector.tensor_copy(
                            out=vat[:tlen, 2 * t:2 * t + 2, 0:64],
                            in_=ps[:tlen].rearrange("p (a b) -> p a b", a=2))
                    va[tt] = vat
                for h_ in range(H):
                    r0 = (h_ % 2) * 64
                    dtl = h_ // 2
                    att = pool.tile([P, 2, S], BF16, tag="att", bufs=4, name="att")
                    for tt in range(2):
                        tlen = TLEN[tt]
                        ps = psum.tile([P, S], F32, tag="sc", bufs=2)
                        nc.tensor.matmul(ps[:tlen],
                                         kT[r0:r0 + 64, dtl, tt * P:tt * P + tlen],
                                         qT[r0:r0 + 64, dtl, :], start=True, stop=True)
                        nc.scalar.activation(att[:tlen, tt], ps[:tlen], AF.Exp)
                    for st in range(2):
                        slen = TLEN[st]
                        po = psum.tile([P, 65], F32, tag="av", bufs=2)
                        for tt in range(2):
                            tlen = TLEN[tt]
                            nc.tensor.matmul(po[:slen],
                                             att[:tlen, tt, st * P:st * P + slen],
                                             va[tt][:tlen, h_, :],
                                             start=(tt == 0), stop=(tt == 1))
                        rc = pool.tile([P, 1], F32, tag="rc", bufs=2, name="rc")
                        nc.vector.reciprocal(rc[:slen], po[:slen, 64:65])
                        o_dst = o_t[2 * i + st][:slen, 64 * h_:64 * h_ + 64]
                        if h_ % 2 == 0:
                            nc.scalar.activation(o_dst, po[:slen, 0:64], AF.Identity,
                                                 scale=rc[:slen])
                        else:
                            nc.vector.tensor_scalar_mul(o_dst, po[:slen, 0:64], rc[:slen])
            for t in range(8 if not os.environ.get("SKIP_ATTN") else 0):
                slen = TLEN[t % 2]
                nc.vector.tensor_tensor(xt[t][:slen], xt[t][:slen], o_t[t][:slen], ALU.add)

            # ---- MLP ----
            hT2 = layernorm_transpose("ln2")
            if l + 1 < L:
                wcur = load_weights(l + 1)
            hT2v = hT2.rearrange("p k (i c) -> p k i c", c=256)
            for g in range(2 if not os.environ.get("SKIP_MLP") else 0):
                m = pool.tile([P, MK, 2 * S], BF16, tag="m", bufs=1, name="m")
                for mt in range(MK):
                    ps = psum.tile([P, 2 * S], F32, tag="m1", bufs=2)
                    for k in range(DK):
                        nc.tensor.matmul(ps[:], w1t[:, k, mt * P:(mt + 1) * P],
                                         hT2v[:, k, 2 * g:2 * g + 2, 0:S],
                                         start=(k == 0), stop=(k == DK - 1))
                    nc.scalar.activation(m[:, mt], ps[:], AF.Gelu,
                                         bias=b1p[:, mt:mt + 1])
                for sj in range(4):  # s-tiles of this group: img 2g+sj//2, half sj%2
                    xti = 2 * (2 * g + sj // 2) + (sj % 2)
                    slen = TLEN[sj % 2]
                    s0 = (sj // 2) * S + (sj % 2) * P
                    for hf in range(2):
                        ps = psum.tile([P, 384], F32, tag="A", bufs=2)
                        for kt in range(MK):
                            nc.tensor.matmul(ps[:slen], m[:, kt, s0:s0 + slen],
                                             w2t[:, kt, hf * 384:(hf + 1) * 384],
                                             start=(kt == 0), stop=False)
                        nc.tensor.matmul(ps[:slen], ones_b[:, :slen],
                                         b2row[:, hf * 384:(hf + 1) * 384],
                                         start=False, stop=True)
                        xs = xt[xti][:slen, hf * 384:(hf + 1) * 384]
                        nc.vector.tensor_tensor(xs, ps[:slen], xs, ALU.add)

        # ---- head: logits for the 4 cls rows ----
        with ExitStack() as hctx:
            hpool = hctx.enter_context(tc.tile_pool(name="hp", bufs=1))
            whT = hpool.tile([P, DK, 1000], BF16, tag="whT")
            nc.sync.dma_start(whT[:], WhT_d.rearrange("k p m -> p k m"))
            bhrow = hpool.tile([1, 1000], BF16, tag="bhrow")
            nc.sync.dma_start(bhrow[:], bh_d[:, :])
            x0T = hpool.tile([P, DK, BIMG], F32, tag="x0T")
            for i in range(BIMG):
                for c in range(DK):
                    nc.sync.dma_start(x0T[:, c, i:i + 1],
                                      xt[2 * i + 1][68:69, c * P:(c + 1) * P])
            x0Tb = hpool.tile([P, DK, BIMG], BF16, tag="x0Tb")
            nc.vector.tensor_copy(out=x0Tb[:], in_=x0T[:])
            outsb = hpool.tile([BIMG, 1000], F32, tag="outsb")
            for hf in range(2):
                ps = psum.tile([BIMG, 500], F32, tag="A", bufs=2)
                for k in range(DK):
                    nc.tensor.matmul(ps[:], x0Tb[:, k], whT[:, k, hf * 500:(hf + 1) * 500],
                                     start=(k == 0), stop=False)
                nc.tensor.matmul(ps[:], ones_b[:, :BIMG], bhrow[:, hf * 500:(hf + 1) * 500],
                                 start=False, stop=True)
                nc.vector.tensor_copy(out=outsb[:, hf * 500:(hf + 1) * 500], in_=ps[:])
            nc.sync.dma_start(out_d[:, :], outsb[:])
    nc.compile()
    return nc


_NC_CACHE = None


def _prep(inputs):
    """Host-side: shard batch, patchify, fold LN gamma/beta + biases + 1/8 scale."""
    f = {k: np.asarray(v, np.float32) for k, v in inputs.items()}
    g1, b1g = f["ln1_g"], f["ln1_b"]
    g2, b2g = f["ln2_g"], f["ln2_b"]

    # block-diagonal per-head QKV, gamma folded, q scaled by 1/8
    def bd(W, gamma, scale):
        # W [L,H,DH,DH] (e,d); out [L,DK,P,P]: blk[l,t,dl,el] = W[l,h,el%64,dl%64]*g[l,d]
        out = np.zeros((L, DK, P, P), np.float32)
        for t in range(DK):
            for j in range(2):
                h = 2 * t + j
                w = W[:, h] * gamma[:, None, 64 * h:64 * h + 64]  # [L,e,d]
                out[:, t, 64 * j:64 * j + 64, 64 * j:64 * j + 64] = \
                    np.transpose(w, (0, 2, 1)) * scale
        return out
    Wqk = np.stack([bd(f["Wq"], g1, 0.125), bd(f["Wk"], g1, 1.0)], 1)  # [L,2,DK,P,P]
    Wv = bd(f["Wv"], g1, 1.0)
    # biases: b' = b + W @ beta_head ; q scaled by 1/8; layout [L,3,DK,P]
    bqkv = np.zeros((L, 3, DK, P), np.float32)
    for jj, (W, b, sc) in enumerate([(f["Wq"], f["bq"], 0.125),
                                     (f["Wk"], f["bk"], 1.0),
                                     (f["Wv"], f["bv"], 1.0)]):
        for h in range(H):
            be = (b[:, h] + np.einsum("led,ld->le", W[:, h],
                                      b1g[:, 64 * h:64 * h + 64])) * sc
            bqkv[:, jj, h // 2, 64 * (h % 2):64 * (h % 2) + 64] = be
    W1T = np.ascontiguousarray(
        (f["W1"] * g2[:, None, :]).transpose(0, 2, 1).reshape(L, DK, P, HID))
    b1 = (f["b1"] + np.einsum("lmd,ld->lm", f["W1"], b2g)).reshape(L, MK, P)
    W2T = np.ascontiguousarray(f["W2"].transpose(0, 2, 1).reshape(L, MK, P, D))
    b2 = f["b2"].reshape(L, 1, D)
    WhT = np.ascontiguousarray(f["Wh"].T.reshape(DK, P, 1000))

    pos = _pos_embed()
    base = np.zeros((BIMG, 2, P, D), np.float32)
    for i in range(BIMG):
        base[i, 0] = pos[1:129]                       # patches 0..127
        base[i, 1, :68] = pos[129:197]                # patches 128..195
        base[i, 1, 68] = pos[0] + f["cls"][0]         # cls token last
    base = base.reshape(8, P, D)

    img = f["image"].reshape(NCORES, BIMG, 3, 14, 16, 14, 16)
    # patchesT [DK,P,BIMG*196]: d=(c,p,q); column = img*196 + (i*14+j)
    pat = img.transpose(0, 2, 4, 6, 1, 3, 5).reshape(NCORES, 768, BIMG, NPATCH)
    pat = pat.reshape(NCORES, DK, P, BIMG * NPATCH)

    shared = {
        "base_x": base, "WeT": f["W_embed"].T.reshape(DK, P, D).astype(BF),
        "be_row": f["b_embed"].reshape(1, D).astype(BF),
        "Wqk_d": Wqk.astype(BF), "Wv_d": Wv.astype(BF), "bqkv_d": bqkv,
        "W1T_d": W1T.astype(BF), "b1_d": b1, "W2T_d": W2T.astype(BF),
        "b2_d": b2.astype(BF), "WhT_d": WhT.astype(BF),
        "bv_row_d": bqkv[:, 2].reshape(L, 1, D).astype(BF),
        "bh_d": f["bh"].reshape(1, 1000).astype(BF),
    }
    return [dict(shared, patchesT=pat[c].astype(BF)) for c in range(NCORES)]


def _run(inputs, trace=False):
    global _NC_CACHE
    if _NC_CACHE is None:
        _NC_CACHE = _build()
    in_maps = _prep(inputs)
    res = run_bass_kernel_spmd(_NC_CACHE, in_maps, core_ids=list(range(NCORES)),
                               trace=trace)
    out = np.concatenate([res.results[c]["out"] for c in range(NCORES)], 0)
    return out.astype(np.float32), res


def kernel(**inputs):
    return _run(inputs, trace=False)[0]
